# revision 1
# baseline (speedup 1.0000x reference)
"""Trainium2 Bass kernel for nn_Adapter (conv1x1 -> LN -> maxpool4x4 -> MLP ->
maxunpool -> deconv1x1 -> residual), data-parallel over batch on 8 NeuronCores.

Self-contained: hardcodes shapes B=32, C=768, H=W=64; shards batch 4-per-core.

Dataflow (per core, 4 batch images).  The kernel is HBM-roofline bound
(~100 MB per core); everything is organized to keep the SDMA queues fed:
  - x is stored in SBUF as bf16, cast inline by the gpsimd/SWDGE in-DMA:
    3 x 4 MB chunk-pair reads per batch into an 8-slot pool (~2.7 batches
    resident), so the next batches' input streams have free slots during the
    current batch's middle chain.
  - conv C->1 in bf16: per 512-col tile j, 6 accumulating matmuls into PSUM
    rows packed at partitions {0,32,64} of 3 banks (the three col groups run
    concurrently on the PE).
  - LayerNorm / maxpool / equality-mask unpool in the [8-part, 512-free]
    layout, fp32.  MLP weights are loaded with NATURAL-layout DMAs and
    transposed once at setup via PE-transpose (a transposed DRAM read would
    shatter into 16k 4-byte descriptors and flood the DMA engines); all
    other param loads ride the scalar ring + engine casts so the SWDGE ring
    carries only bulk x casts from t=0.
  - out = (dw[p] * unp[f]) + x (+ dcb): with deconv_b == 0 (runtime-checked)
    the unp row is broadcast to 128 partitions ONCE per batch (8 ones (x) unp
    matmuls -> PSUM -> ScalarE copies -> u_sb bf16), and each out 512-tile is
    one all-SBUF DVE scalar_tensor_tensor (u*dw[p] + x).  PE: 8 matmuls per
    batch instead of 48.
  - software pipelining: batch b's out phase is emitted during iteration b+1
    (after the b+2 input stream is queued), so in-DMAs always precede the
    out-DMAs that could head-of-line block them; in-DMAs ride the SWDGE
    ring, 1 MB out-DMAs the sync ring, small middle-chain DMAs the scalar
    ring.
Numerics: out ~= x + (tiny rank-1 term); bf16 x contributes ~1.7e-3 rel err
(gate is 2e-2).  conv_b is skipped (LayerNorm shift-invariance cancels it).
"""
import sys
import numpy as np

if '/opt/trn_rl_repo' not in sys.path:
    sys.path.insert(0, '/opt/trn_rl_repo')

B, C, H, W = 32, 768, 64, 64
HW = H * W          # 4096
NCORES = 8
NB = B // NCORES    # 4 batches per core
NCH = C // 128      # 6 C-chunks
NJ = HW // 512      # 8 column tiles

_CACHE = {}


def _build_nc(ln_trivial=False, dcb_trivial=False):
    import concourse.bass as bass
    import concourse.bacc as bacc
    import concourse.tile as tile
    from concourse import mybir

    f32 = mybir.dt.float32
    bf16 = mybir.dt.bfloat16
    AluOp = mybir.AluOpType
    Act = mybir.ActivationFunctionType

    nc = bacc.Bacc("TRN2", target_bir_lowering=False, debug=False,
                   num_devices=NCORES)

    x_d = nc.declare_dram_parameter("x", [NB, C, H, W], f32, isOutput=False)
    cw_d = nc.declare_dram_parameter("conv_w", [C], f32, isOutput=False)
    nc.declare_dram_parameter("conv_b", [1], f32, isOutput=False)
    lg_d = nc.declare_dram_parameter("ln_g", [W], f32, isOutput=False)
    lb_d = nc.declare_dram_parameter("ln_b", [W], f32, isOutput=False)
    dw_d = nc.declare_dram_parameter("down_w", [64, 256], f32, isOutput=False)
    db_d = nc.declare_dram_parameter("down_b", [64], f32, isOutput=False)
    uw_d = nc.declare_dram_parameter("up_w", [256, 64], f32, isOutput=False)
    ub_d = nc.declare_dram_parameter("up_b", [256], f32, isOutput=False)
    dcw_d = nc.declare_dram_parameter("deconv_w", [C], f32, isOutput=False)
    dcb_d = nc.declare_dram_parameter("deconv_b", [C], f32, isOutput=False)
    out_d = nc.declare_dram_parameter("out", [NB, C, H, W], f32, isOutput=True)

    with tile.TileContext(nc) as tc:
        with (
            tc.tile_pool(name="xp", bufs=9) as xp,
            tc.tile_pool(name="op", bufs=3) as op,
            tc.tile_pool(name="ub", bufs=1) as ub_pool,
            tc.tile_pool(name="sg", bufs=1) as sg,
            tc.tile_pool(name="sm", bufs=1) as sm,
            tc.tile_pool(name="unp", bufs=1) as unp_pool,
            tc.tile_pool(name="ps_y", bufs=1, space="PSUM") as ps_y,
            tc.tile_pool(name="ps_u", bufs=3, space="PSUM") as ps_u,
            tc.tile_pool(name="ps_m", bufs=1, space="PSUM") as ps_m,
        ):
            # ---------------- one-time parameter staging ----------------
            # all param loads ride the scalar (q10) ring + engine casts so
            # the SWDGE ring carries ONLY the bulk x casts from t=0
            cw32 = sg.tile([128, NCH], f32, tag="cw32")
            nc.scalar.dma_start(
                out=cw32, in_=cw_d.ap().rearrange("(k p) -> p k", p=128))
            w_sb = sg.tile([128, NCH], bf16, tag="w")       # conv_w chunks
            nc.scalar.copy(out=w_sb, in_=cw32)
            dcb_sb = sg.tile([128, NCH], f32, tag="dcb")    # deconv_b chunks
            nc.scalar.dma_start(
                out=dcb_sb, in_=dcb_d.ap().rearrange("(k p) -> p k", p=128))
            # deconv_w: bf16 row for the general-path outer products, and
            # bf16 per-chunk columns for the fast-path per-partition scale
            if not dcb_trivial:
                dwr32 = sg.tile([1, C], f32, tag="dwr32")
                nc.scalar.dma_start(out=dwr32, in_=dcw_d.ap().unsqueeze(0))
                dw_row = sg.tile([1, C], bf16, tag="dwrow")
                nc.scalar.copy(out=dw_row, in_=dwr32)
            dcw32 = sg.tile([128, NCH], f32, tag="dcw32")
            nc.scalar.dma_start(
                out=dcw32, in_=dcw_d.ap().rearrange("(k p) -> p k", p=128))
            dw_sb = sg.tile([128, NCH], bf16, tag="dwsb")
            nc.scalar.copy(out=dw_sb, in_=dcw32)
            ones_row = sg.tile([1, 128], bf16, tag="ones")
            nc.vector.memset(ones_row, 1.0)

            xts_all = [[] for _ in range(NB)]

            def emit_in(bi):
                # the full 12 MB input read for batch bi, as 3 x 4 MB
                # chunk-pair DMAs with an inline f32 -> bf16 cast (SWDGE
                # ring); xts_all keeps per-chunk [128, 4096] views
                xr = x_d.ap()[bi].rearrange("(g p) h w -> p g (h w)", p=128)
                for k in range(NCH // 2):
                    xt = xp.tile([128, 2 * HW], bf16, tag="x")
                    nc.gpsimd.dma_start(
                        out=xt.rearrange("p (g hw) -> p g hw", g=2),
                        in_=xr[:, 2 * k:2 * k + 2, :])
                    xts_all[bi].append(xt[:, 0:HW])
                    xts_all[bi].append(xt[:, HW:2 * HW])

            # start the batch-0 input stream immediately; nothing precedes
            # it on the SWDGE ring.
            emit_in(0)

            # identity for PE-transposes: ident[p, f] = (f - p == 0)
            iraw = sg.tile([128, 128], f32, tag="iraw")
            nc.gpsimd.iota(out=iraw, pattern=[[1, 128]], base=0,
                           channel_multiplier=-1,
                           allow_small_or_imprecise_dtypes=True)
            izero = sg.tile([128, 1], f32, tag="izero")
            nc.vector.memset(izero, 0.0)
            ident = iraw
            nc.vector.tensor_tensor(
                out=ident, in0=iraw,
                in1=izero.to_broadcast([128, 128]), op=AluOp.is_equal)

            # MLP weights: natural-layout DMA loads + PE transpose (see doc)
            down_nat = sg.tile([64, 256], f32, tag="dnat")
            nc.scalar.dma_start(out=down_nat, in_=dw_d.ap())
            up_nat = sg.tile([128, 128], f32, tag="unat")
            for k in range(2):
                nc.scalar.dma_start(out=up_nat[:, k * 64:(k + 1) * 64],
                                    in_=uw_d.ap()[k * 128:(k + 1) * 128, :])
            down_wT = sg.tile([128, 128], f32, tag="dwT")   # [256,64]T chunks
            for k in range(2):
                tp = ps_u.tile([128, 64], f32, tag="u")
                nc.tensor.transpose(
                    out=tp, in_=down_nat[:, k * 128:(k + 1) * 128],
                    identity=ident[0:64, 0:64])
                nc.scalar.copy(out=down_wT[:, k * 64:(k + 1) * 64], in_=tp)
            up_wT = sg.tile([64, 256], f32, tag="uwT")      # [64, 256]
            for k in range(2):
                tp = ps_u.tile([64, 128], f32, tag="u")
                nc.tensor.transpose(
                    out=tp, in_=up_nat[:, k * 64:(k + 1) * 64],
                    identity=ident)
                nc.scalar.copy(out=up_wT[:, k * 128:(k + 1) * 128], in_=tp)

            dnb_sb = sg.tile([64, 1], f32, tag="dnb")
            nc.scalar.dma_start(out=dnb_sb, in_=db_d.ap().unsqueeze(1))
            ub_sb = sg.tile([128, 2], f32, tag="ub")
            nc.scalar.dma_start(
                out=ub_sb, in_=ub_d.ap().rearrange("(k p) -> p k", p=128))

            # ln_g / ln_b replicated into the [8, h_sub, w] layout (general
            # LN path only; the trivial path never touches them)
            if not ln_trivial:
                g8 = sg.tile([8, 8, 64], f32, tag="g8")
                nc.scalar.dma_start(
                    out=g8,
                    in_=lg_d.ap().unsqueeze(0).unsqueeze(0)
                    .to_broadcast([8, 8, 64]))
                g8n = sg.tile([8, 8, 64], f32, tag="g8n")
                nc.scalar.mul(out=g8n, in_=g8, mul=-1.0)    # negated ln_g
                b8 = sg.tile([8, 8, 64], f32, tag="b8")
                nc.scalar.dma_start(
                    out=b8,
                    in_=lb_d.ap().unsqueeze(0).unsqueeze(0)
                    .to_broadcast([8, 8, 64]))
            eps8 = sg.tile([8, 1], f32, tag="eps8")
            nc.vector.memset(eps8, 1e-5)

            # ---------------- per-batch pipeline ----------------
            u_sb_all = [None] * NB

            def out_phase(b):
                # out = (dw[p] * unp[f]) + x (+ dcb), fp32 half-chunk tiles,
                # 1 MB out-DMAs on the sync (SP HWDGE) ring: separate queue
                # from the SWDGE in-casts, and fine enough that the final
                # drain chases the STTs closely.
                xts = xts_all[b]
                if dcb_trivial:
                    u_sb = u_sb_all[b]
                    for c in range(NCH):
                        for h in range(2):
                            ot = op.tile([128, HW // 2], f32, tag="o")
                            for jj in range(NJ // 2):
                                j = h * 4 + jj
                                nc.vector.scalar_tensor_tensor(
                                    out=ot[:, jj * 512:(jj + 1) * 512],
                                    in0=u_sb[:, j * 512:(j + 1) * 512],
                                    scalar=dw_sb[:, c:c + 1],
                                    in1=xts[c][:, j * 512:(j + 1) * 512],
                                    op0=AluOp.mult, op1=AluOp.add)
                            nc.sync.dma_start(
                                out=out_d.ap()[b, c * 128:(c + 1) * 128]
                                .rearrange("p h w -> p (h w)")
                                [:, h * (HW // 2):(h + 1) * (HW // 2)],
                                in_=ot)
                else:
                    unp_row = unp_all[b]
                    for c in range(NCH):
                        for h in range(2):
                            ot = op.tile([128, HW // 2], f32, tag="o")
                            for jj in range(NJ // 2):
                                j = h * 4 + jj
                                u_ps = ps_u.tile([128, 512], f32, tag="u")
                                nc.tensor.matmul(
                                    out=u_ps,
                                    lhsT=dw_row[0:1, c * 128:(c + 1) * 128],
                                    rhs=unp_row[0:1, j * 512:(j + 1) * 512],
                                    start=True, stop=True)
                                nc.vector.scalar_tensor_tensor(
                                    out=ot[:, jj * 512:(jj + 1) * 512],
                                    in0=u_ps,
                                    scalar=dcb_sb[:, c:c + 1],
                                    in1=xts[c][:, j * 512:(j + 1) * 512],
                                    op0=AluOp.add, op1=AluOp.add)
                            nc.sync.dma_start(
                                out=out_d.ap()[b, c * 128:(c + 1) * 128]
                                .rearrange("p h w -> p (h w)")
                                [:, h * (HW // 2):(h + 1) * (HW // 2)],
                                in_=ot)

            unp_all = [None] * NB

            for b in range(NB):
                xts = xts_all[b]

                # conv C->1 in bf16.  Loop c-outer so matmuls issue in
                # chunk-arrival order and the PE streams densely behind the
                # DMA.  The 8 accumulator groups live in 3 PSUM banks, packed
                # at base partitions {0, 32, 64} (engine-legal offsets).
                y_tiles = []
                for t in range(3):
                    y_t = ps_y.tile([65, 512], f32, tag=f"y{t}")
                    y_tiles.append(y_t)
                ypos = [(j // 3, 32 * (j % 3)) for j in range(NJ)]
                for c in range(NCH):
                    for j in range(NJ):
                        t, p0 = ypos[j]
                        nc.tensor.matmul(
                            out=y_tiles[t][p0:p0 + 1, :],
                            lhsT=w_sb[:, c:c + 1],
                            rhs=xts[c][:, j * 512:(j + 1) * 512],
                            start=(c == 0), stop=(c == NCH - 1))

                # queue the whole next-batch input stream now: slots are free
                # and these precede the previous batch's out-DMAs in SWDGE
                # ring order, so the in-stream can never be head-of-line
                # blocked by an out-DMA waiting on compute.
                if b + 1 < NB:
                    emit_in(b + 1)

                # software pipelining: the previous batch's out phase is
                # emitted HERE, after the next input stream is queued, so the
                # DMA engines stay fed through this batch's middle chain.
                if b > 0:
                    out_phase(b - 1)

                # Stage the 8 [1,512] results side by side on partition 0,
                # then scatter to [8, 512] (engine writes can't target
                # partitions 1..7 directly).
                # y staged in bf16 (LN stats accumulate in fp32; yl/pooled
                # stay fp32 so the unpool equality mask is exact)
                y_row = sm.tile([1, HW], bf16, tag="yrow")
                y8 = sm.tile([8, 512], bf16, tag="y8b")
                yrv = y_row.rearrange("p (j w) -> p j w", j=8)
                for half in range(2):
                    for j in range(4 * half, 4 * half + 4):
                        t, p0 = ypos[j]
                        nc.scalar.copy(
                            out=y_row[0:1, j * 512:(j + 1) * 512],
                            in_=y_tiles[t][p0:p0 + 1, :])
                    nc.scalar.dma_start(
                        out=y8[4 * half:4 * half + 4],
                        in_=yrv[:, 4 * half:4 * half + 4])

                # LayerNorm over W in the [8, h_sub, w] layout (h = 8j+h_sub)
                y3 = y8.rearrange("j (hs w) -> j hs w", hs=8)
                ysq = sm.tile([8, 512], bf16, tag="ysq")
                nc.scalar.square(out=ysq, in_=y8)           # parallel to DVE
                musum = sm.tile([8, 8], f32, tag="musum")
                nc.vector.reduce_sum(out=musum, in_=y3, axis=mybir.AxisListType.X)
                sumsq = sm.tile([8, 8], f32, tag="sumsq")
                nc.vector.reduce_sum(out=sumsq,
                                     in_=ysq.rearrange("j (hs w) -> j hs w", hs=8),
                                     axis=mybir.AxisListType.X)
                m2 = sm.tile([8, 8], f32, tag="m2")
                nc.vector.tensor_mul(m2, musum, musum)
                # v = m2/64 - sumsq = -64*var ; sd = sqrt(-v/64 + eps)
                v8 = sm.tile([8, 8], f32, tag="v8")
                nc.vector.scalar_tensor_tensor(
                    out=v8, in0=m2, scalar=1.0 / 64.0, in1=sumsq,
                    op0=AluOp.mult, op1=AluOp.subtract)
                sd = sm.tile([8, 8], f32, tag="sd")
                nc.scalar.activation(out=sd, in_=v8, func=Act.Sqrt,
                                     bias=eps8, scale=-1.0 / 64.0)
                tneg = sm.tile([8, 8, 64], bf16, tag="tneg")  # mu - y
                mu_bc = musum.unsqueeze(2).to_broadcast([8, 8, 64])
                nc.vector.scalar_tensor_tensor(
                    out=tneg, in0=mu_bc, scalar=1.0 / 64.0, in1=y3,
                    op0=AluOp.mult, op1=AluOp.subtract)
                rstd = sm.tile([8, 8], f32, tag="rstd")
                nc.vector.reciprocal(out=rstd, in_=sd)
                if ln_trivial:
                    # ln_g == 1, ln_b == 0 (checked at runtime in kernel()):
                    # yl = (y-mu)*rstd = tneg * (-rstd)
                    rstdn = sm.tile([8, 8], f32, tag="rstdn")
                    nc.scalar.mul(out=rstdn, in_=rstd, mul=-1.0)
                    yl = sm.tile([8, 8, 64], f32, tag="yl")
                    rn_bc = rstdn.unsqueeze(2).to_broadcast([8, 8, 64])
                    nc.vector.tensor_mul(yl, tneg, rn_bc)
                else:
                    # yl = (y-mu)*rstd*g + b  ==  tneg*rstd*(-g) + b
                    t2 = sm.tile([8, 8, 64], f32, tag="t2")
                    rstd_bc = rstd.unsqueeze(2).to_broadcast([8, 8, 64])
                    nc.vector.tensor_mul(t2, tneg, rstd_bc)
                    t3 = sm.tile([8, 8, 64], f32, tag="t3")
                    nc.vector.tensor_mul(t3, t2, g8n)
                    yl = sm.tile([8, 8, 64], f32, tag="yl")
                    nc.vector.tensor_add(yl, t3, b8)

                # maxpool 4x4 in two steps, all APs <= 4 dims.
                # hs = 4*hp2 + hin; w = 4*wp + win; hp = 2j + hp2
                colmax = sm.tile([8, 8, 16], f32, tag="colmax")  # (hs, wp)
                nc.vector.reduce_max(
                    out=colmax,
                    in_=yl.rearrange("j hs (wp win) -> j hs wp win", win=4),
                    axis=mybir.AxisListType.X)
                pooled = sm.tile([8, 2, 16], f32, tag="pooled")  # (hp2, wp)
                nc.vector.reduce_max(
                    out=pooled,
                    in_=colmax.rearrange("j (hp2 hin) wp -> j hp2 wp hin",
                                         hp2=2),
                    axis=mybir.AxisListType.X)

                # MLP: flat [256] -> relu(down) [64] -> up [256]
                flat_sb = sm.tile([128, 2], f32, tag="flat")
                for k in range(2):
                    nc.scalar.dma_start(out=flat_sb[:, k:k + 1],
                                        in_=pooled[4 * k:4 * k + 4])
                down_ps = ps_m.tile([64, 1], f32, tag="down")
                for k in range(2):
                    nc.tensor.matmul(out=down_ps,
                                     lhsT=down_wT[:, k * 64:(k + 1) * 64],
                                     rhs=flat_sb[:, k:k + 1],
                                     start=(k == 0), stop=(k == 1))
                down_sb = sm.tile([64, 1], f32, tag="down_sb")
                nc.scalar.activation(out=down_sb, in_=down_ps, func=Act.Relu,
                                     bias=dnb_sb, scale=1.0)
                up_ps = ps_m.tile([128, 2], f32, tag="up")
                for k in range(2):
                    nc.tensor.matmul(out=up_ps[:, k:k + 1],
                                     lhsT=up_wT[:, k * 128:(k + 1) * 128],
                                     rhs=down_sb, start=True, stop=True)
                up_sb = sm.tile([128, 2], f32, tag="up_sb")
                for k in range(2):
                    nc.scalar.activation(out=up_sb[:, k:k + 1],
                                         in_=up_ps[:, k:k + 1],
                                         func=Act.Identity,
                                         bias=ub_sb[:, k:k + 1], scale=1.0)
                up8 = sm.tile([8, 2, 16], f32, tag="up8")
                for k in range(2):
                    nc.scalar.dma_start(out=up8[4 * k:4 * k + 4],
                                        in_=up_sb[:, k:k + 1])

                # unpool: expand pooled and up to the [8, hs, w] layout in two
                # broadcast-copy steps each (keeps every AP <= 4 dims), then
                # mask = (yl == pooled_x), unp = mask * up_x (written bf16).
                pooled_h = sm.tile([8, 8, 16], f32, tag="pooled_h")  # (hs, wp)
                nc.vector.tensor_copy(
                    out=pooled_h.rearrange("j (hp2 hin) wp -> j hp2 hin wp",
                                           hp2=2),
                    in_=pooled.unsqueeze(2).to_broadcast([8, 2, 4, 16]))
                pooled_x = sm.tile([8, 8, 64], f32, tag="px")
                nc.vector.tensor_copy(
                    out=pooled_x.rearrange("j hs (wp win) -> j (hs wp) win",
                                           win=4),
                    in_=(pooled_h.rearrange("j hs wp -> j (hs wp)")
                         .unsqueeze(2).to_broadcast([8, 128, 4])))
                up_h = sm.tile([8, 8, 16], f32, tag="pooled_h")
                nc.vector.tensor_copy(
                    out=up_h.rearrange("j (hp2 hin) wp -> j hp2 hin wp",
                                       hp2=2),
                    in_=up8.unsqueeze(2).to_broadcast([8, 2, 4, 16]))
                up_x = sm.tile([8, 8, 64], bf16, tag="up_x")
                nc.vector.tensor_copy(
                    out=up_x.rearrange("j hs (wp win) -> j (hs wp) win", win=4),
                    in_=(up_h.rearrange("j hs wp -> j (hs wp)")
                         .unsqueeze(2).to_broadcast([8, 128, 4])))

                mask8 = sm.tile([8, 8, 64], bf16, tag="ysq")
                nc.vector.tensor_tensor(out=mask8, in0=yl, in1=pooled_x,
                                        op=AluOp.is_equal)
                unp8 = sm.tile([8, 8, 64], bf16, tag="unp8")
                nc.vector.tensor_mul(unp8, mask8, up_x)

                # unp as one bf16 [1, 4096] row (matmul rhs starts at part 0);
                # plain HWDGE DMA on the ACT ring.  Natural (h, w) raster.
                unp_row = unp_pool.tile([1, HW], bf16, tag="row")
                nc.scalar.dma_start(
                    out=unp_row.rearrange("p (j hsw) -> p j hsw", j=8),
                    in_=unp8)
                unp_all[b] = unp_row

                if dcb_trivial:
                    # broadcast unp to all 128 partitions once: 8 ones (x) unp
                    # matmuls -> PSUM -> ScalarE copies -> u_sb bf16.  The out
                    # phase itself is emitted next iteration (see out_phase).
                    u_sb = ub_pool.tile([128, HW], bf16, tag="u")
                    for j in range(NJ):
                        u_ps = ps_u.tile([128, 512], f32, tag="u")
                        nc.tensor.matmul(
                            out=u_ps, lhsT=ones_row,
                            rhs=unp_row[0:1, j * 512:(j + 1) * 512],
                            start=True, stop=True)
                        nc.scalar.copy(out=u_sb[:, j * 512:(j + 1) * 512],
                                       in_=u_ps)
                    u_sb_all[b] = u_sb

            out_phase(NB - 1)

    nc.compile()
    return nc


def _get_nc(**kw):
    key = tuple(sorted(kw.items()))
    if key not in _CACHE:
        _CACHE[key] = _build_nc(**kw)
    return _CACHE[key]


def _make_in_maps(inputs):
    x = np.ascontiguousarray(np.asarray(inputs["x"], dtype=np.float32))
    params = {k: np.ascontiguousarray(np.asarray(v, dtype=np.float32))
              for k, v in inputs.items() if k != "x"}
    in_maps = []
    for core in range(NCORES):
        m = {"x": np.ascontiguousarray(x[core * NB:(core + 1) * NB])}
        m.update(params)
        in_maps.append(m)
    return in_maps


def _run(inputs, trace=False, **build_kw):
    from concourse.bass_utils import run_bass_kernel_spmd
    if 'ln_trivial' not in build_kw:
        build_kw['ln_trivial'] = bool(
            np.all(np.asarray(inputs['ln_g']) == 1.0)
            and np.all(np.asarray(inputs['ln_b']) == 0.0))
    if 'dcb_trivial' not in build_kw:
        build_kw['dcb_trivial'] = bool(
            np.all(np.asarray(inputs['deconv_b']) == 0.0))
    nc = _get_nc(**build_kw)
    in_maps = _make_in_maps(inputs)
    res = run_bass_kernel_spmd(nc, in_maps, core_ids=list(range(NCORES)),
                               trace=trace)
    out = np.concatenate([res.results[c]["out"] for c in range(NCORES)], axis=0)
    return out, res


def kernel(**inputs) -> np.ndarray:
    out, _ = _run(inputs)
    return out



# revision 12
# speedup vs baseline: 1.1366x; 1.1366x over previous
"""Trainium2 Bass kernel for nn_Adapter (conv1x1 -> LN -> maxpool4x4 -> MLP ->
maxunpool -> deconv1x1 -> residual), data-parallel over batch on 8 NeuronCores.

Self-contained: hardcodes shapes B=32, C=768, H=W=64; shards batch 4-per-core.

Dataflow (per core, 4 batch images).  The kernel is HBM-roofline bound
(~75 MB per core: 50 MB fp32 x read + 25 MB bf16 out write); everything is
organized to keep the SDMA queues fed:
  - x is stored in SBUF as bf16, cast inline by the gpsimd/SWDGE in-DMA:
    6 x 2 MB per-chunk reads per batch into a 17-slot pool (~2.8 batches
    resident), so the next batches' input streams have free slots during the
    current batch's middle chain, and each slot recycles as soon as its
    chunk's out-phase STT is done.
  - conv C->1 in bf16: per 512-col tile j, 6 accumulating matmuls into PSUM
    rows packed at partitions {0,32,64} of 3 banks (the three col groups run
    concurrently on the PE).
  - LayerNorm / maxpool / equality-mask unpool in the [8-part, 512-free]
    layout, fp32.  MLP weights are loaded with NATURAL-layout DMAs and
    transposed once at setup via PE-transpose (a transposed DRAM read would
    shatter into 16k 4-byte descriptors and flood the DMA engines); all
    other param loads ride the scalar ring + engine casts so the SWDGE ring
    carries only bulk x casts from t=0.
  - out = (dw[p] * unp[f]) + x (+ dcb): with deconv_b == 0 (runtime-checked)
    the unp row is broadcast to 128 partitions ONCE per batch (8 ones (x) unp
    matmuls -> PSUM -> Scalar/Vector copies -> u_sb bf16), and each out chunk
    is ONE all-SBUF DVE scalar_tensor_tensor (u*dw[p] + x) over 4096 cols,
    written bf16 (the host upcasts to f32 while gathering; the rank-1 term is
    ~0.1% of |x| so bf16 rounding costs ~1e-3 rel err against a 2e-2 gate).
  - software pipelining: batch b's out phase is emitted during iteration b+1
    (after the b+2 input stream is queued), so in-DMAs always precede the
    out-DMAs that could head-of-line block them; in-DMAs ride the SWDGE
    ring, 1 MB bf16 out-DMAs the sync ring, small middle-chain DMAs the
    scalar ring.
Numerics: out ~= x + (tiny rank-1 term); bf16 x + bf16 out contribute ~2e-3
rel err (gate is 2e-2).  conv_b is skipped (LN shift-invariance cancels it).
"""
import sys
import numpy as np

if '/opt/trn_rl_repo' not in sys.path:
    sys.path.insert(0, '/opt/trn_rl_repo')

B, C, H, W = 32, 768, 64, 64
HW = H * W          # 4096
NCORES = 8
NB = B // NCORES    # 4 batches per core
NCH = C // 128      # 6 C-chunks
NJ = HW // 512      # 8 column tiles

_CACHE = {}


def _build_nc(ln_trivial=False, dcb_trivial=False):
    import concourse.bass as bass
    import concourse.bacc as bacc
    import concourse.tile as tile
    from concourse import mybir

    f32 = mybir.dt.float32
    bf16 = mybir.dt.bfloat16
    AluOp = mybir.AluOpType
    Act = mybir.ActivationFunctionType

    nc = bacc.Bacc("TRN2", target_bir_lowering=False, debug=False,
                   num_devices=NCORES)

    x_d = nc.declare_dram_parameter("x", [NB, C, H, W], f32, isOutput=False)
    cw_d = nc.declare_dram_parameter("conv_w", [C], f32, isOutput=False)
    nc.declare_dram_parameter("conv_b", [1], f32, isOutput=False)
    lg_d = nc.declare_dram_parameter("ln_g", [W], f32, isOutput=False)
    lb_d = nc.declare_dram_parameter("ln_b", [W], f32, isOutput=False)
    dw_d = nc.declare_dram_parameter("down_w", [64, 256], f32, isOutput=False)
    db_d = nc.declare_dram_parameter("down_b", [64], f32, isOutput=False)
    uw_d = nc.declare_dram_parameter("up_w", [256, 64], f32, isOutput=False)
    ub_d = nc.declare_dram_parameter("up_b", [256], f32, isOutput=False)
    dcw_d = nc.declare_dram_parameter("deconv_w", [C], f32, isOutput=False)
    dcb_d = nc.declare_dram_parameter("deconv_b", [C], f32, isOutput=False)
    out_d = nc.declare_dram_parameter("out", [NB, C, H, W], bf16, isOutput=True)

    with tile.TileContext(nc) as tc:
        with (
            tc.tile_pool(name="xp", bufs=17) as xp,
            tc.tile_pool(name="op", bufs=3) as op,
            tc.tile_pool(name="ub", bufs=2) as ub_pool,
            tc.tile_pool(name="sg", bufs=1) as sg,
            tc.tile_pool(name="sm", bufs=1) as sm,
            tc.tile_pool(name="unp", bufs=1) as unp_pool,
            tc.tile_pool(name="ps_y", bufs=1, space="PSUM") as ps_y,
            tc.tile_pool(name="ps_u", bufs=3, space="PSUM") as ps_u,
            tc.tile_pool(name="ps_m", bufs=1, space="PSUM") as ps_m,
        ):
            xts_all = [[] for _ in range(NB)]

            def emit_in(bi):
                # the full 12 MB input read for batch bi, as 6 x 2 MB
                # per-chunk DMAs with an inline f32 -> bf16 cast (SWDGE
                # ring); per-chunk tiles recycle as soon as that chunk's
                # out-phase STT has consumed them.
                xr = x_d.ap()[bi].rearrange("(g p) h w -> g p (h w)", p=128)
                for k in range(NCH):
                    xt = xp.tile([128, HW], bf16, tag="x")
                    nc.gpsimd.dma_start(out=xt, in_=xr[k])
                    xts_all[bi].append(xt)

            # start the batch-0 input stream immediately; nothing precedes
            # it on the SWDGE ring.
            emit_in(0)

            # ---------------- one-time parameter staging ----------------
            # all param loads ride the scalar (q10) ring + engine casts so
            # the SWDGE ring carries ONLY the bulk x casts from t=0
            cw32 = sg.tile([128, NCH], f32, tag="cw32")
            nc.scalar.dma_start(
                out=cw32, in_=cw_d.ap().rearrange("(k p) -> p k", p=128))
            w_sb = sg.tile([128, NCH], bf16, tag="w")       # conv_w chunks
            nc.scalar.copy(out=w_sb, in_=cw32)
            dcb_sb = sg.tile([128, NCH], f32, tag="dcb")    # deconv_b chunks
            nc.scalar.dma_start(
                out=dcb_sb, in_=dcb_d.ap().rearrange("(k p) -> p k", p=128))
            # deconv_w: bf16 row for the general-path outer products, and
            # bf16 per-chunk columns for the fast-path per-partition scale
            if not dcb_trivial:
                dwr32 = sg.tile([1, C], f32, tag="dwr32")
                nc.scalar.dma_start(out=dwr32, in_=dcw_d.ap().unsqueeze(0))
                dw_row = sg.tile([1, C], bf16, tag="dwrow")
                nc.scalar.copy(out=dw_row, in_=dwr32)
            dcw32 = sg.tile([128, NCH], f32, tag="dcw32")
            nc.scalar.dma_start(
                out=dcw32, in_=dcw_d.ap().rearrange("(k p) -> p k", p=128))
            dw_sb = sg.tile([128, NCH], bf16, tag="dwsb")
            nc.scalar.copy(out=dw_sb, in_=dcw32)
            ones_row = sg.tile([1, 128], bf16, tag="ones")
            nc.vector.memset(ones_row, 1.0)

            # identity for PE-transposes: ident[p, f] = (f - p == 0)
            iraw = sg.tile([128, 128], f32, tag="iraw")
            nc.gpsimd.iota(out=iraw, pattern=[[1, 128]], base=0,
                           channel_multiplier=-1,
                           allow_small_or_imprecise_dtypes=True)
            izero = sg.tile([128, 1], f32, tag="izero")
            nc.vector.memset(izero, 0.0)
            ident = iraw
            nc.vector.tensor_tensor(
                out=ident, in0=iraw,
                in1=izero.to_broadcast([128, 128]), op=AluOp.is_equal)

            # MLP weights: natural-layout DMA loads + PE transpose (see doc)
            down_nat = sg.tile([64, 256], f32, tag="dnat")
            nc.scalar.dma_start(out=down_nat, in_=dw_d.ap())
            up_nat = sg.tile([128, 128], f32, tag="unat")
            for k in range(2):
                nc.scalar.dma_start(out=up_nat[:, k * 64:(k + 1) * 64],
                                    in_=uw_d.ap()[k * 128:(k + 1) * 128, :])
            down_wT = sg.tile([128, 128], f32, tag="dwT")   # [256,64]T chunks
            for k in range(2):
                tp = ps_u.tile([128, 64], f32, tag="u")
                nc.tensor.transpose(
                    out=tp, in_=down_nat[:, k * 128:(k + 1) * 128],
                    identity=ident[0:64, 0:64])
                nc.scalar.copy(out=down_wT[:, k * 64:(k + 1) * 64], in_=tp)
            up_wT = sg.tile([64, 256], f32, tag="uwT")      # [64, 256]
            for k in range(2):
                tp = ps_u.tile([64, 128], f32, tag="u")
                nc.tensor.transpose(
                    out=tp, in_=up_nat[:, k * 64:(k + 1) * 64],
                    identity=ident)
                nc.scalar.copy(out=up_wT[:, k * 128:(k + 1) * 128], in_=tp)

            dnb_sb = sg.tile([64, 1], f32, tag="dnb")
            nc.scalar.dma_start(out=dnb_sb, in_=db_d.ap().unsqueeze(1))
            ub_sb = sg.tile([128, 2], f32, tag="ub")
            nc.scalar.dma_start(
                out=ub_sb, in_=ub_d.ap().rearrange("(k p) -> p k", p=128))

            # ln_g / ln_b replicated into the [8, h_sub, w] layout (general
            # LN path only; the trivial path never touches them)
            if not ln_trivial:
                g8 = sg.tile([8, 8, 64], f32, tag="g8")
                nc.scalar.dma_start(
                    out=g8,
                    in_=lg_d.ap().unsqueeze(0).unsqueeze(0)
                    .to_broadcast([8, 8, 64]))
                g8n = sg.tile([8, 8, 64], f32, tag="g8n")
                nc.scalar.mul(out=g8n, in_=g8, mul=-1.0)    # negated ln_g
                b8 = sg.tile([8, 8, 64], f32, tag="b8")
                nc.scalar.dma_start(
                    out=b8,
                    in_=lb_d.ap().unsqueeze(0).unsqueeze(0)
                    .to_broadcast([8, 8, 64]))
            eps8 = sg.tile([8, 1], f32, tag="eps8")
            nc.vector.memset(eps8, 1e-5)

            # ---------------- per-batch pipeline ----------------
            u_sb_all = [None] * NB

            def out_phase(b):
                # out = (dw[p] * unp[f]) + x (+ dcb), bf16 full-chunk tiles,
                # 1 MB out-DMAs on the sync (SP HWDGE) ring: separate queue
                # from the SWDGE in-casts; one STT + one DMA per chunk.
                xts = xts_all[b]
                if dcb_trivial:
                    u_sb = u_sb_all[b]
                    for c in range(NCH):
                        ot = op.tile([128, HW], bf16, tag="o")
                        nc.vector.scalar_tensor_tensor(
                            out=ot, in0=u_sb, scalar=dw_sb[:, c:c + 1],
                            in1=xts[c], op0=AluOp.mult, op1=AluOp.add)
                        nc.sync.dma_start(
                            out=out_d.ap()[b, c * 128:(c + 1) * 128]
                            .rearrange("p h w -> p (h w)"),
                            in_=ot)
                else:
                    unp_row = unp_all[b]
                    for c in range(NCH):
                        ot = op.tile([128, HW], bf16, tag="o")
                        for j in range(NJ):
                            u_ps = ps_u.tile([128, 512], f32, tag="u")
                            nc.tensor.matmul(
                                out=u_ps,
                                lhsT=dw_row[0:1, c * 128:(c + 1) * 128],
                                rhs=unp_row[0:1, j * 512:(j + 1) * 512],
                                start=True, stop=True)
                            nc.vector.scalar_tensor_tensor(
                                out=ot[:, j * 512:(j + 1) * 512],
                                in0=u_ps,
                                scalar=dcb_sb[:, c:c + 1],
                                in1=xts[c][:, j * 512:(j + 1) * 512],
                                op0=AluOp.add, op1=AluOp.add)
                        nc.sync.dma_start(
                            out=out_d.ap()[b, c * 128:(c + 1) * 128]
                            .rearrange("p h w -> p (h w)"),
                            in_=ot)

            unp_all = [None] * NB

            for b in range(NB):
                xts = xts_all[b]

                # conv C->1 in bf16.  Loop c-outer so matmuls issue in
                # chunk-arrival order and the PE streams densely behind the
                # DMA.  The 8 accumulator groups live in 3 PSUM banks, packed
                # at base partitions {0, 32, 64} (engine-legal offsets).
                y_tiles = []
                for t in range(3):
                    y_t = ps_y.tile([65, 512], f32, tag=f"y{t}")
                    y_tiles.append(y_t)
                ypos = [(j // 3, 32 * (j % 3)) for j in range(NJ)]
                for c in range(NCH):
                    for j in range(NJ):
                        t, p0 = ypos[j]
                        nc.tensor.matmul(
                            out=y_tiles[t][p0:p0 + 1, :],
                            lhsT=w_sb[:, c:c + 1],
                            rhs=xts[c][:, j * 512:(j + 1) * 512],
                            start=(c == 0), stop=(c == NCH - 1))

                # queue the whole next-batch input stream now: slots are free
                # and these precede the previous batch's out-DMAs in SWDGE
                # ring order, so the in-stream can never be head-of-line
                # blocked by an out-DMA waiting on compute.
                if b + 1 < NB:
                    emit_in(b + 1)

                # software pipelining: the previous batch's out phase is
                # emitted HERE, after the next input stream is queued, so the
                # DMA engines stay fed through this batch's middle chain.
                if b > 0:
                    out_phase(b - 1)

                # Stage the 8 [1,512] results side by side on partition 0,
                # then scatter to [8, 512] (engine writes can't target
                # partitions 1..7 directly).
                # y staged in bf16 (LN stats accumulate in fp32; yl/pooled
                # stay fp32 so the unpool equality mask is exact)
                y_row = sm.tile([1, HW], bf16, tag="yrow")
                y8 = sm.tile([8, 512], bf16, tag="y8b")
                yrv = y_row.rearrange("p (j w) -> p j w", j=8)
                for half in range(2):
                    for j in range(4 * half, 4 * half + 4):
                        t, p0 = ypos[j]
                        nc.scalar.copy(
                            out=y_row[0:1, j * 512:(j + 1) * 512],
                            in_=y_tiles[t][p0:p0 + 1, :])
                    nc.scalar.dma_start(
                        out=y8[4 * half:4 * half + 4],
                        in_=yrv[:, 4 * half:4 * half + 4])

                # LayerNorm over W in the [8, h_sub, w] layout (h = 8j+h_sub)
                y3 = y8.rearrange("j (hs w) -> j hs w", hs=8)
                ysq = sm.tile([8, 512], bf16, tag="ysq")
                nc.scalar.square(out=ysq, in_=y8)           # parallel to DVE
                musum = sm.tile([8, 8], f32, tag="musum")
                nc.vector.reduce_sum(out=musum, in_=y3, axis=mybir.AxisListType.X)
                sumsq = sm.tile([8, 8], f32, tag="sumsq")
                nc.vector.reduce_sum(out=sumsq,
                                     in_=ysq.rearrange("j (hs w) -> j hs w", hs=8),
                                     axis=mybir.AxisListType.X)
                m2 = sm.tile([8, 8], f32, tag="m2")
                nc.vector.tensor_mul(m2, musum, musum)
                # v = m2/64 - sumsq = -64*var ; sd = sqrt(-v/64 + eps)
                v8 = sm.tile([8, 8], f32, tag="v8")
                nc.vector.scalar_tensor_tensor(
                    out=v8, in0=m2, scalar=1.0 / 64.0, in1=sumsq,
                    op0=AluOp.mult, op1=AluOp.subtract)
                sd = sm.tile([8, 8], f32, tag="sd")
                nc.scalar.activation(out=sd, in_=v8, func=Act.Sqrt,
                                     bias=eps8, scale=-1.0 / 64.0)
                tneg = sm.tile([8, 8, 64], bf16, tag="tneg")  # mu - y
                mu_bc = musum.unsqueeze(2).to_broadcast([8, 8, 64])
                nc.vector.scalar_tensor_tensor(
                    out=tneg, in0=mu_bc, scalar=1.0 / 64.0, in1=y3,
                    op0=AluOp.mult, op1=AluOp.subtract)
                rstd = sm.tile([8, 8], f32, tag="rstd")
                nc.vector.reciprocal(out=rstd, in_=sd)
                if ln_trivial:
                    # ln_g == 1, ln_b == 0 (checked at runtime in kernel()):
                    # yl = (y-mu)*rstd = tneg * (-rstd)
                    rstdn = sm.tile([8, 8], f32, tag="rstdn")
                    nc.scalar.mul(out=rstdn, in_=rstd, mul=-1.0)
                    yl = sm.tile([8, 8, 64], f32, tag="yl")
                    rn_bc = rstdn.unsqueeze(2).to_broadcast([8, 8, 64])
                    nc.vector.tensor_mul(yl, tneg, rn_bc)
                else:
                    # yl = (y-mu)*rstd*g + b  ==  tneg*rstd*(-g) + b
                    t2 = sm.tile([8, 8, 64], f32, tag="t2")
                    rstd_bc = rstd.unsqueeze(2).to_broadcast([8, 8, 64])
                    nc.vector.tensor_mul(t2, tneg, rstd_bc)
                    t3 = sm.tile([8, 8, 64], f32, tag="t3")
                    nc.vector.tensor_mul(t3, t2, g8n)
                    yl = sm.tile([8, 8, 64], f32, tag="yl")
                    nc.vector.tensor_add(yl, t3, b8)

                # maxpool 4x4 in two steps, all APs <= 4 dims.
                # hs = 4*hp2 + hin; w = 4*wp + win; hp = 2j + hp2
                colmax = sm.tile([8, 8, 16], f32, tag="colmax")  # (hs, wp)
                nc.vector.reduce_max(
                    out=colmax,
                    in_=yl.rearrange("j hs (wp win) -> j hs wp win", win=4),
                    axis=mybir.AxisListType.X)
                pooled = sm.tile([8, 2, 16], f32, tag="pooled")  # (hp2, wp)
                nc.vector.reduce_max(
                    out=pooled,
                    in_=colmax.rearrange("j (hp2 hin) wp -> j hp2 wp hin",
                                         hp2=2),
                    axis=mybir.AxisListType.X)

                # MLP: flat [256] -> relu(down) [64] -> up [256]
                flat_sb = sm.tile([128, 2], f32, tag="flat")
                for k in range(2):
                    nc.scalar.dma_start(out=flat_sb[:, k:k + 1],
                                        in_=pooled[4 * k:4 * k + 4])
                down_ps = ps_m.tile([64, 1], f32, tag="down")
                for k in range(2):
                    nc.tensor.matmul(out=down_ps,
                                     lhsT=down_wT[:, k * 64:(k + 1) * 64],
                                     rhs=flat_sb[:, k:k + 1],
                                     start=(k == 0), stop=(k == 1))
                down_sb = sm.tile([64, 1], f32, tag="down_sb")
                nc.scalar.activation(out=down_sb, in_=down_ps, func=Act.Relu,
                                     bias=dnb_sb, scale=1.0)
                up_ps = ps_m.tile([128, 2], f32, tag="up")
                for k in range(2):
                    nc.tensor.matmul(out=up_ps[:, k:k + 1],
                                     lhsT=up_wT[:, k * 128:(k + 1) * 128],
                                     rhs=down_sb, start=True, stop=True)
                up_sb = sm.tile([128, 2], f32, tag="up_sb")
                for k in range(2):
                    nc.scalar.activation(out=up_sb[:, k:k + 1],
                                         in_=up_ps[:, k:k + 1],
                                         func=Act.Identity,
                                         bias=ub_sb[:, k:k + 1], scale=1.0)
                up8 = sm.tile([8, 2, 16], f32, tag="up8")
                for k in range(2):
                    nc.scalar.dma_start(out=up8[4 * k:4 * k + 4],
                                        in_=up_sb[:, k:k + 1])

                # unpool: expand pooled and up to the [8, hs, w] layout in two
                # broadcast-copy steps each (keeps every AP <= 4 dims), then
                # mask = (yl == pooled_x), unp = mask * up_x (written bf16).
                pooled_h = sm.tile([8, 8, 16], f32, tag="pooled_h")  # (hs, wp)
                nc.vector.tensor_copy(
                    out=pooled_h.rearrange("j (hp2 hin) wp -> j hp2 hin wp",
                                           hp2=2),
                    in_=pooled.unsqueeze(2).to_broadcast([8, 2, 4, 16]))
                pooled_x = sm.tile([8, 8, 64], f32, tag="px")
                nc.vector.tensor_copy(
                    out=pooled_x.rearrange("j hs (wp win) -> j (hs wp) win",
                                           win=4),
                    in_=(pooled_h.rearrange("j hs wp -> j (hs wp)")
                         .unsqueeze(2).to_broadcast([8, 128, 4])))
                up_h = sm.tile([8, 8, 16], f32, tag="pooled_h")
                nc.vector.tensor_copy(
                    out=up_h.rearrange("j (hp2 hin) wp -> j hp2 hin wp",
                                       hp2=2),
                    in_=up8.unsqueeze(2).to_broadcast([8, 2, 4, 16]))
                up_x = sm.tile([8, 8, 64], bf16, tag="up_x")
                nc.vector.tensor_copy(
                    out=up_x.rearrange("j hs (wp win) -> j (hs wp) win", win=4),
                    in_=(up_h.rearrange("j hs wp -> j (hs wp)")
                         .unsqueeze(2).to_broadcast([8, 128, 4])))

                mask8 = sm.tile([8, 8, 64], bf16, tag="ysq")
                nc.vector.tensor_tensor(out=mask8, in0=yl, in1=pooled_x,
                                        op=AluOp.is_equal)
                unp8 = sm.tile([8, 8, 64], bf16, tag="unp8")
                nc.vector.tensor_mul(unp8, mask8, up_x)

                # unp as one bf16 [1, 4096] row (matmul rhs starts at part 0);
                # plain HWDGE DMA on the ACT ring.  Natural (h, w) raster.
                unp_row = unp_pool.tile([1, HW], bf16, tag="row")
                nc.scalar.dma_start(
                    out=unp_row.rearrange("p (j hsw) -> p j hsw", j=8),
                    in_=unp8)
                unp_all[b] = unp_row

                if dcb_trivial:
                    # broadcast unp to all 128 partitions once: 8 ones (x) unp
                    # matmuls -> PSUM -> Scalar/Vector copies (alternating, to
                    # halve the serial latency of this stage) -> u_sb bf16.
                    # The out phase itself is emitted next iteration.
                    u_sb = ub_pool.tile([128, HW], bf16, tag="u")
                    for j in range(NJ):
                        u_ps = ps_u.tile([128, 512], f32, tag="u")
                        nc.tensor.matmul(
                            out=u_ps, lhsT=ones_row,
                            rhs=unp_row[0:1, j * 512:(j + 1) * 512],
                            start=True, stop=True)
                        if j % 2 == 0:
                            nc.scalar.copy(
                                out=u_sb[:, j * 512:(j + 1) * 512], in_=u_ps)
                        else:
                            nc.vector.tensor_copy(
                                out=u_sb[:, j * 512:(j + 1) * 512], in_=u_ps)
                    u_sb_all[b] = u_sb

            out_phase(NB - 1)

    nc.compile()
    return nc


def _get_nc(**kw):
    key = tuple(sorted(kw.items()))
    if key not in _CACHE:
        _CACHE[key] = _build_nc(**kw)
    return _CACHE[key]


def _make_in_maps(inputs):
    x = np.ascontiguousarray(np.asarray(inputs["x"], dtype=np.float32))
    params = {k: np.ascontiguousarray(np.asarray(v, dtype=np.float32))
              for k, v in inputs.items() if k != "x"}
    in_maps = []
    for core in range(NCORES):
        m = {"x": np.ascontiguousarray(x[core * NB:(core + 1) * NB])}
        m.update(params)
        in_maps.append(m)
    return in_maps


def _run(inputs, trace=False, **build_kw):
    from concourse.bass_utils import run_bass_kernel_spmd
    if 'ln_trivial' not in build_kw:
        build_kw['ln_trivial'] = bool(
            np.all(np.asarray(inputs['ln_g']) == 1.0)
            and np.all(np.asarray(inputs['ln_b']) == 0.0))
    if 'dcb_trivial' not in build_kw:
        build_kw['dcb_trivial'] = bool(
            np.all(np.asarray(inputs['deconv_b']) == 0.0))
    nc = _get_nc(**build_kw)
    in_maps = _make_in_maps(inputs)
    res = run_bass_kernel_spmd(nc, in_maps, core_ids=list(range(NCORES)),
                               trace=trace)
    # device writes bf16; upcast to f32 while gathering the batch shards
    out = np.concatenate(
        [np.asarray(res.results[c]["out"]).astype(np.float32)
         for c in range(NCORES)], axis=0)
    return out, res


def kernel(**inputs) -> np.ndarray:
    out, _ = _run(inputs)
    return out

